# revision 8
# baseline (speedup 1.0000x reference)
"""CrossReferenceAttention Trainium2 kernel.

Reference computation (per batch b):
    scores[i,j] = M[ids[i], ids[j]] if ids[i]>0 and ids[j]>0 else 0
    attn = softmax(scores / sqrt(H), axis=-1)      # masked entries exp(0)=1
    ctx = attn @ hidden ; proj = ctx @ W^T + b
    out = LN(hidden + proj) * ln_w + ln_b

Strategy: data-parallel over batch (B=8 -> 8 NeuronCores). Masking is folded
into index redirection: masked i gathers a zero row of the padded matrix,
masked j gathers a zero row of the transposed scratch, so exp(0)=1 falls out
naturally with no mask tensors.

Per core dataflow (i split into 4 subranges of 256 for pipelining):
  1. transpose-mode dma_gather: rows M[idx_i] land column-major in SBUF
     (rt_sb[p, c, i] = M[idx_i[i], c*128+p]) as bf16.
  2. indirect scatter SBUF->DRAM with per-column indices; columns never used
     by any j are redirected out-of-bounds and skipped -> only ~1K of 8320
     columns are written to the compact scratch rt[8321, 256].
  3. plain dma_gather rows rt[idx_j] -> scoresT[j%128, j//128, i] in SBUF:
     exactly the [j-partition, i-free] layout the PE matmuls need.
  4. expT = exp(scoresT / sqrt(H)) on ScalarE (bf16).
  5. denom[i] = sum_j expT via ones-stationary matmuls (PSUM accumulate).
  6. ctxT[d,i] = sum_j hid[j,d] * expT[j,i] on PE (bf16, f32 accumulate).
  7. proj[i,:] = ctxT[:,i]^T @ W^T, normalized by 1/denom[i] (diag scaling
     commutes through both matmuls), + bias, + hidden, LayerNorm, store.
"""

import math

import ml_dtypes
import numpy as np

import concourse.bacc as bacc
import concourse.bass as bass
import concourse.mybir as mybir
import concourse.tile as tile
from concourse.tile_rust import add_dep_helper

P = 128
B = 8
S = 1024           # sequence length
H = 768            # hidden dim
V = 8192           # max verse id
NSUB = 4           # i-subranges per core
ISUB = S // NSUB   # 256
NCH = S // P       # 8 j-chunks (and i-tiles)
DCH = H // P       # 6 d-chunks
MROWS = V + 2      # 8194: rows of padded matrix (8193 real + zero row)
MCOLS = 8320       # padded columns (65 * 128)
CCH = MCOLS // P   # 65
RT_ROWS = MCOLS + 1  # 8321: transposed scratch rows (last = zero row)
SCALE = 1.0 / math.sqrt(H)
OOB = 1 << 20

F32 = mybir.dt.float32
BF16 = mybir.dt.bfloat16
I16 = mybir.dt.int16
I32 = mybir.dt.int32

_CACHED_NC = None


def build_nc():
    """Build (and cache) the single-core Bass program shared by all 8 cores."""
    global _CACHED_NC
    if _CACHED_NC is not None:
        return _CACHED_NC

    # 32KB DMA descriptor carveout (2048 descs): two 128-idx transpose
    # gathers (522 descs each) plus a column scatter (520) and a j-gather
    # (65) can be in flight on the single SWDGE queue concurrently.
    nc = bacc.Bacc(
        "TRN2",
        target_bir_lowering=False,
        debug=False,
        dynamic_dma_scratch_size=32768,
    )

    m_pad = nc.dram_tensor("m_pad", [MROWS, MCOLS], BF16, kind="ExternalInput")
    hid = nc.dram_tensor("hid", [S, H], F32, kind="ExternalInput")
    w_t = nc.dram_tensor("w_t", [H, H], BF16, kind="ExternalInput")
    idx_i = nc.dram_tensor("idx_i", [P, S // 16], I16, kind="ExternalInput")
    idx_j = nc.dram_tensor("idx_j", [P, S // 16], I16, kind="ExternalInput")
    scat = nc.dram_tensor("scat", [P, CCH], I32, kind="ExternalInput")
    b3 = nc.dram_tensor("b3", [3, H], F32, kind="ExternalInput")  # out_b, ln_w, ln_b
    out = nc.dram_tensor("out", [S, H], F32, kind="ExternalOutput")
    rts = [nc.dram_tensor(f"rt{s}", [RT_ROWS, ISUB], BF16) for s in range(NSUB)]

    hid_r = hid.ap().rearrange("(c p) d -> p c d", p=P)
    w_r = w_t.ap().rearrange("(c p) o -> p c o", p=P)
    out_r = out.ap().rearrange("(t p) d -> p t d", p=P)

    with tile.TileContext(nc) as tc:
        with (
            tc.tile_pool(name="singles", bufs=1) as singles,
            tc.tile_pool(name="gather", bufs=3) as gpool,
            tc.tile_pool(name="scores", bufs=2) as spool,
            tc.tile_pool(name="ctx", bufs=2) as cpool,
            tc.tile_pool(name="epi", bufs=3) as epool,
            tc.tile_pool(name="dpsum", bufs=1, space="PSUM") as dpsum,
            tc.tile_pool(name="cpsum", bufs=2, space="PSUM") as cpsum,
            tc.tile_pool(name="ppsum", bufs=2, space="PSUM") as ppsum,
        ):
            # ---- one-time loads -------------------------------------------
            idxi_sb = singles.tile([P, S // 16], I16)
            nc.sync.dma_start(out=idxi_sb[:], in_=idx_i.ap())
            idxj_sb = singles.tile([P, S // 16], I16)
            nc.sync.dma_start(out=idxj_sb[:], in_=idx_j.ap())
            scat_sb = singles.tile([P, CCH], I32)
            nc.sync.dma_start(out=scat_sb[:], in_=scat.ap())

            hid_f32 = singles.tile([P, NCH, H], F32)
            nc.sync.dma_start(out=hid_f32[:], in_=hid_r)
            hid_bf = singles.tile([P, NCH, H], BF16)
            nc.scalar.activation(
                out=hid_bf[:], in_=hid_f32[:],
                func=mybir.ActivationFunctionType.Copy,
            )
            w_sb = singles.tile([P, DCH, H], BF16)
            nc.sync.dma_start(out=w_sb[:], in_=w_r)

            # out_b / ln_w / ln_b broadcast to all partitions
            b_rep = singles.tile([P, 3, H], F32)
            nc.gpsimd.dma_start(
                out=b_rep[:],
                in_=bass.AP(b3, 0, [[0, P], [H, 3], [1, H]]),
            )

            ones_bf = singles.tile([P, 1], BF16)
            nc.vector.memset(ones_bf[:], 1.0)
            eps_sb = singles.tile([P, 1], F32)
            nc.vector.memset(eps_sb[:], 1e-5)
            zrow = singles.tile([1, ISUB], BF16)
            nc.vector.memset(zrow[:], 0.0)

            # ---- per i-subrange pipeline ----------------------------------
            for s in range(NSUB):
                # 1+2. transpose-gather rows of m_pad column-major into SBUF
                # (128 idxs per gather to fit the descriptor ring), then
                # OOB-filtered column scatter into the compact scratch.
                zw = nc.sync.dma_start(
                    out=rts[s].ap()[RT_ROWS - 1 : RT_ROWS, :], in_=zrow[:1, :]
                )
                scs = []
                for h in range(2):
                    hh = 2 * s + h  # global 128-idx chunk
                    rt_sb = gpool.tile([P, CCH, P], BF16)
                    nc.gpsimd.dma_gather(
                        out_ap=rt_sb[:],
                        in_ap=m_pad.ap(),
                        idxs_ap=idxi_sb[:, 8 * hh : 8 * (hh + 1)],
                        num_idxs=P,
                        num_idxs_reg=P,
                        elem_size=MCOLS,
                        transpose=True,
                    )
                    sc = nc.gpsimd.indirect_dma_start(
                        out=rts[s].ap(),
                        out_offset=bass.IndirectOffsetOnAxis(ap=scat_sb[:], axis=0),
                        in_=rt_sb[:],
                        in_offset=None,
                        element_offset=P * h,
                        bounds_check=RT_ROWS - 1,
                        oob_is_err=False,
                    )
                    scs.append(sc)

                # 3. gather scoresT[j%128, j//128, i] = rt[idx_j[j], :]
                sco = spool.tile([P, NCH, ISUB], BF16)
                g3 = nc.gpsimd.dma_gather(
                    out_ap=sco[:],
                    in_ap=rts[s].ap(),
                    idxs_ap=idxj_sb[:],
                    num_idxs=S,
                    num_idxs_reg=S,
                    elem_size=ISUB,
                    transpose=False,
                )
                # DRAM RAW: gather must follow the scatters + zero-row write
                for sc in scs:
                    add_dep_helper(g3.ins, sc.ins, reason="rt scatter->gather RAW")
                add_dep_helper(g3.ins, zw.ins, reason="rt zero row->gather RAW")

                # 4. expT = exp(scoresT / sqrt(H))
                ex = spool.tile([P, NCH, ISUB], BF16)
                nc.scalar.activation(
                    out=ex[:], in_=sco[:],
                    func=mybir.ActivationFunctionType.Exp,
                    scale=SCALE,
                )

                # 5. denom per i-tile: den[i%128, u] = sum_j expT[j, i]
                # (expT tile as bf16 stationary, ones as moving -> [128, 1];
                # avoids fp32 PE matmuls, which diverge on HW)
                den_ps = dpsum.tile([P, 2], F32)
                for u in range(2):
                    for c in range(NCH):
                        nc.tensor.matmul(
                            den_ps[:, u : u + 1],
                            lhsT=ex[:, c, P * u : P * (u + 1)],
                            rhs=ones_bf[:, :1],
                            start=(c == 0), stop=(c == NCH - 1),
                        )
                rec = epool.tile([P, 2], F32)
                nc.vector.reciprocal(out=rec[:], in_=den_ps[:])

                # 6. ctxT[d, i] = sum_j hid_bf[j, d] * expT[j, i]
                cts = cpool.tile([P, DCH, ISUB], BF16)
                for d in range(DCH):
                    cps = cpsum.tile([P, ISUB], F32)
                    for c in range(NCH):
                        nc.tensor.matmul(
                            cps[:],
                            lhsT=hid_bf[:, c, P * d : P * (d + 1)],
                            rhs=ex[:, c, :],
                            start=(c == 0), stop=(c == NCH - 1),
                        )
                    nc.vector.tensor_copy(out=cts[:, d, :], in_=cps[:])

                # 7. proj + epilogue per i-tile of 128
                for u in range(2):
                    t_idx = 2 * s + u
                    pps = ppsum.tile([P, H], F32)
                    for d in range(DCH):
                        nc.tensor.matmul(
                            pps[:, 0:512],
                            lhsT=cts[:, d, P * u : P * (u + 1)],
                            rhs=w_sb[:, d, 0:512],
                            start=(d == 0), stop=(d == DCH - 1),
                        )
                        nc.tensor.matmul(
                            pps[:, 512:H],
                            lhsT=cts[:, d, P * u : P * (u + 1)],
                            rhs=w_sb[:, d, 512:H],
                            start=(d == 0), stop=(d == DCH - 1),
                        )

                    res = epool.tile([P, H], F32)
                    nc.vector.tensor_scalar_mul(
                        out=res[:], in0=pps[:], scalar1=rec[:, u : u + 1]
                    )
                    nc.vector.tensor_add(out=res[:], in0=res[:], in1=b_rep[:, 0, :])
                    nc.vector.tensor_add(
                        out=res[:], in0=res[:], in1=hid_f32[:, t_idx, :]
                    )

                    # LayerNorm over H=768 (bn_stats in 3 chunks of 256)
                    st = epool.tile([P, 3, 6], F32)
                    for g in range(3):
                        nc.vector.bn_stats(
                            out=st[:, g, :], in_=res[:, 256 * g : 256 * (g + 1)]
                        )
                    mv = epool.tile([P, 2], F32)
                    nc.vector.bn_aggr(out=mv[:], in_=st[:])
                    rstd = epool.tile([P, 1], F32)
                    nc.scalar.activation(
                        out=rstd[:], in_=mv[:, 1:2],
                        func=mybir.ActivationFunctionType.Sqrt,
                        bias=eps_sb[:], scale=1.0,
                    )
                    nc.vector.reciprocal(out=rstd[:], in_=rstd[:])
                    o1 = epool.tile([P, H], F32)
                    nc.vector.tensor_scalar(
                        out=o1[:], in0=res[:],
                        scalar1=mv[:, 0:1], scalar2=rstd[:],
                        op0=mybir.AluOpType.subtract, op1=mybir.AluOpType.mult,
                    )
                    nc.vector.tensor_mul(out=o1[:], in0=o1[:], in1=b_rep[:, 1, :])
                    nc.vector.tensor_add(out=o1[:], in0=o1[:], in1=b_rep[:, 2, :])
                    nc.sync.dma_start(out=out_r[:, t_idx, :], in_=o1[:])

    nc.compile()
    _CACHED_NC = nc
    return nc


def _wrap16(v):
    """Lay out 1024 indices the way SWDGE gather kernels read them:
    unwrapped[k] = tile[k % 16, k // 16], replicated across the 8 q7 cores."""
    w = np.ascontiguousarray(v.reshape(-1, 16).T)  # [16, 64]
    return np.tile(w, (P // 16, 1)).astype(np.int16)


def prep_inputs(hidden_states, verse_ids, cross_ref_matrix, out_w, out_b, ln_w, ln_b):
    """Host-side staging: pad/cast the matrix, build gather/scatter indices."""
    m_bf = np.zeros((MROWS, MCOLS), dtype=ml_dtypes.bfloat16)
    m_bf[: V + 1, : V + 1] = np.asarray(cross_ref_matrix).astype(ml_dtypes.bfloat16)

    w_t = np.ascontiguousarray(np.asarray(out_w).T).astype(ml_dtypes.bfloat16)
    b3 = np.stack(
        [
            np.asarray(out_b, np.float32),
            np.asarray(ln_w, np.float32),
            np.asarray(ln_b, np.float32),
        ]
    )

    in_maps = []
    for b in range(B):
        ids = np.asarray(verse_ids[b]).astype(np.int64)
        has = ids > 0
        ii = np.where(has, ids, V + 1)       # masked i -> zero row of m_pad
        jj = np.where(has, ids, RT_ROWS - 1)  # masked j -> zero row of rt

        scat = np.full(MCOLS, OOB, np.int32)
        cols = np.unique(ids[has])
        scat[cols] = cols
        scat_pc = np.ascontiguousarray(scat.reshape(CCH, P).T)

        in_maps.append(
            {
                "m_pad": m_bf,
                "hid": np.ascontiguousarray(hidden_states[b], dtype=np.float32),
                "w_t": w_t,
                "idx_i": _wrap16(ii),
                "idx_j": _wrap16(jj),
                "scat": scat_pc,
                "b3": b3,
            }
        )
    return in_maps


def kernel(hidden_states, verse_ids, cross_ref_matrix, out_w, out_b, ln_w, ln_b,
           _trace=False):
    from concourse.bass_utils import run_bass_kernel_spmd

    nc = build_nc()
    in_maps = prep_inputs(
        hidden_states, verse_ids, cross_ref_matrix, out_w, out_b, ln_w, ln_b
    )
    res = run_bass_kernel_spmd(nc, in_maps, core_ids=list(range(B)), trace=_trace)
    out = np.stack([res.results[b]["out"] for b in range(B)])
    if _trace:
        kernel.last_results = res
    return out.astype(np.float32)
